# revision 49
# baseline (speedup 1.0000x reference)
"""Trainium2 Bass kernel for nn_Attention (batch=8, seq=1024, dim=1024, 16 heads x 64).

Strategy: pure data parallelism — one batch element per NeuronCore (8 cores),
full weights replicated, zero collectives. Per core:
  LayerNorm (f32 stats, interleaved with the first qkv group) -> qkv matmul in
  fp32r -> q RMS-normalized per tile; k left unnormalized (1/|k| folds into
  the per-partition exp scale, batched per group) -> q/k to [c, tok] layout
  via XBAR DMA-transpose (DMA engines, not PE) -> scores in bf16 (K=64) ->
  exp on ScalarE with scale=1/|k_j| (no max subtraction: |s|<=64<88) ->
  attn@v: stationary = prob chunks, moving = v||ones (bf16) -> reciprocal on
  DVE + normalize on Pool -> XBAR transpose back -> out-proj (bf16) in two
  passes: chunk-1 heads run as chunk-2 PE filler, chunk-2 heads at the tail.
Engine discipline: ScalarE runs only exp + LN chains + pre-attention squares
(it paces the attention phase; any small ACT op that waits on a backlogged
engine stalls the exp stream behind it, so mid-attention groups route square
via Pool and defer rsqrt to one batched Ln/Exp per group). Out-stores go
through the Pool SWDGE so the SP HWDGE queue carries only XBAR transposes at
the tail. Every PE idle gap stays under ~3.4us — the cost model's p-state
ramp resets on longer gaps and then charges ~2x for the next 3us of matmuls.
"""
import sys

sys.path.insert(0, '/opt/trn_rl_repo')

import numpy as np
import ml_dtypes
import concourse.bass as bass
import concourse.mybir as mybir
import concourse.tile as tile
from concourse import bacc
from concourse.bass_utils import run_bass_kernel_spmd

f32 = mybir.dt.float32
f32r = mybir.dt.float32r
bf16 = mybir.dt.bfloat16
AX = mybir.AxisListType
ALU = mybir.AluOpType
ACTF = mybir.ActivationFunctionType

N = 1024          # tokens per core
D = 1024          # model dim
H = 16            # heads
C = 64            # head dim
NT = N // 128     # token tiles
DT = D // 128     # dim tiles

LN_EPS = 1e-5
RMS_EPS = 1e-24


def build():
    nc = bacc.Bacc(None)
    x = nc.declare_dram_parameter("x", [N, D], f32, isOutput=False)
    wqkv = nc.declare_dram_parameter("wqkv", [D, 2 * D], f32r, isOutput=False)
    wv = nc.declare_dram_parameter("wv", [D, D], f32r, isOutput=False)
    wout = nc.declare_dram_parameter("wout", [D, D], bf16, isOutput=False)
    g = nc.declare_dram_parameter("g", [128, D], bf16, isOutput=False)
    ident = nc.declare_dram_parameter("ident", [128, 128], f32r, isOutput=False)
    out = nc.declare_dram_parameter("out", [N, D], bf16, isOutput=True)

    with tile.TileContext(nc) as tc:
        with tc.tile_pool(name="persist", bufs=1) as pp, \
             tc.tile_pool(name="big", bufs=1) as bigp, \
             tc.tile_pool(name="wstream", bufs=6) as wsp, \
             tc.tile_pool(name="stageA", bufs=2) as sta, \
             tc.tile_pool(name="stageB", bufs=2) as stb, \
             tc.tile_pool(name="kgp", bufs=2) as kgp, \
             tc.tile_pool(name="qfp", bufs=2) as qfp, \
             tc.tile_pool(name="sqp", bufs=1) as sqp, \
             tc.tile_pool(name="ptpool", bufs=16) as ptp, \
             tc.tile_pool(name="small", bufs=3) as smp, \
             tc.tile_pool(name="osbp", bufs=2) as osbp, \
             tc.tile_pool(name="onpk", bufs=2) as onp, \
             tc.tile_pool(name="osp", bufs=3) as osp, \
             tc.tile_pool(name="ps1024", bufs=2, space="PSUM") as ps1024, \
             tc.tile_pool(name="psb", bufs=2, space="PSUM") as psb, \
             tc.tile_pool(name="tpp", bufs=2, space="PSUM") as tpp:

            # x + all XBAR transposes stream on the sync(SP) HWDGE queue;
            # weights stream on the scalar(ACT) queue; out-stores go through
            # the Pool SWDGE so the tail SP queue carries only transposes.
            id_sb = pp.tile([128, 128], f32r, tag="ident")
            nc.scalar.dma_start(id_sb[:], ident[:])
            gfull_sb = pp.tile([128, D], bf16, tag="gfull")
            nc.scalar.dma_start(gfull_sb[:], g[:])
            def w_dma(grp, quarter, pool=None, tag="wg", eng=None):
                w_sb = (pool or wsp).tile([128, DT // 4, 512], f32r, tag=tag,
                                          name=f"w_{grp}_{quarter}")
                if grp >= 4:
                    src = wv[quarter * 256:(quarter + 1) * 256,
                             (grp - 4) * 512:(grp - 3) * 512]
                else:
                    src = wqkv[quarter * 256:(quarter + 1) * 256,
                               grp * 512:(grp + 1) * 512]
                (eng or nc.scalar).dma_start(
                    w_sb[:], src.rearrange("(ko ki) f -> ki ko f", ki=128))
                return w_sb

            w_tiles = {gg: [] for gg in range(6)}
            x_tiles = []
            for tt in range(NT):
                x_sb = sta.tile([128, D], f32, tag="x_t", name=f"x_{tt}")
                nc.sync.dma_start(x_sb[:], x[tt * 128:(tt + 1) * 128, :])
                x_tiles.append(x_sb)
            eps_ln = pp.tile([128, 1], f32, tag="epsln")
            nc.gpsimd.memset(eps_ln[:], LN_EPS)
            eps_rms = pp.tile([128, 1], f32, tag="epsrms")
            nc.gpsimd.memset(eps_rms[:], RMS_EPS)

            # Weights issue at scheduled points where their ring slot is
            # already free, so a dma_start's slot-wait never parks the ACT
            # sequencer (which would stall the exp stream behind it).
            wout_q = []

            def wo_dma(q, pool=None, tag="wg", eng=None):
                w_sb = (pool or wsp).tile([128, 2, D], bf16, tag=tag,
                                          name=f"wo_{q}")
                (eng or nc.scalar).dma_start(
                    w_sb[:], wout[q * 256:(q + 1) * 256, :]
                    .rearrange("(ko ki) d -> ki ko d", ki=128))
                wout_q.append(w_sb)

            # weights issue at scheduled points where their ring slot is
            # already free, so a dma_start's slot-wait never parks the ACT
            # sequencer for long.
            w_order = [("g", 0, q) for q in range(4)] + \
                      [("g", 2, 0), ("g", 2, 1)] + \
                      [("g", g, q) for g in [4, 1, 3, 5] for q in range(4)] + \
                      [("wo", 0, q) for q in range(4)]
            w_pos = [0]

            def w_issue(n):
                for _ in range(n):
                    if w_pos[0] >= len(w_order):
                        return
                    kind_, g, q = w_order[w_pos[0]]
                    w_pos[0] += 1
                    if kind_ == "g":
                        w_tiles[g].append(w_dma(g, q))
                    else:
                        wo_dma(q)

            w_issue(6)   # w0 x4 + w2_0/1 fill the 6-slot ring
            # w2_2/3 ride the x staging slots (free right after LN), so g2
            # never waits on quarters gated by g0's end
            w_tiles[2].append(w_dma(2, 2, pool=sta, tag="x_t"))
            w_tiles[2].append(w_dma(2, 3, pool=sta, tag="x_t"))

            # Persistent big tensors. xnT owns the single "big" slot and dies
            # after the last qkv matmul; o_acc (proj pass-1 partials) reuses
            # the same slot.
            xnT = bigp.tile([128, DT, N], f32r, tag="big")        # [d, dt, t]
            qnT_ab = [pp.tile([128, 4, N], bf16, tag=f"qnT{c}", name=f"qnT_{c}")
                      for c in range(2)]
            knT_ab = [pp.tile([128, 4, N], bf16, tag=f"knT{c}", name=f"knT_{c}")
                      for c in range(2)]
            ohn = pp.tile([128, 8, N], bf16, tag="ohn")           # [hd, pair, t]
            v_aug = pp.tile([128, NT, H, 65], bf16, tag="vaug")   # [j, jt, h, c|1]
            # per-chunk sum-of-squares of raw k rows [tok, tt, h] and the
            # batched 1/|k| used as exp scale
            ssk = [pp.tile([128, NT, 8], f32, tag=f"ssk{c}", name=f"ssk_{c}")
                   for c in range(2)]
            rk_all = [pp.tile([128, NT, 8], f32, tag=f"rk{c}", name=f"rk_{c}")
                      for c in range(2)]
            osb_box = [None, None]
            pts_box = {}
            oacc_box = [None]

            def ln_tile(tt):
                ts = slice(tt * 128, (tt + 1) * 128)
                x_sb = x_tiles[tt]
                s1 = smp.tile([128, 1], f32, tag="s1", name=f"s1_{tt}")
                nc.vector.tensor_reduce(s1[:], x_sb[:], AX.X, ALU.add)
                xn_t = sta.tile([128, D], f32r, tag="xn_t", name=f"xn_{tt}")
                s2 = smp.tile([128, 1], f32, tag="s2", name=f"s2_{tt}")
                # Square output is scratch: write it into xn_t, which the
                # normalize below overwrites anyway (only accum_out is used)
                nc.scalar.activation(xn_t[:, 0:D], x_sb[:], ACTF.Square,
                                     bias=0.0, scale=1.0, accum_out=s2[:])
                m2 = smp.tile([128, 1], f32, tag="m2", name=f"m2_{tt}")
                nc.gpsimd.tensor_tensor(m2[:], s1[:], s1[:], ALU.mult)
                dvar = smp.tile([128, 1], f32, tag="dvar", name=f"dvar_{tt}")
                nc.gpsimd.tensor_scalar(dvar[:], m2[:], -1.0 / D, s2[:],
                                        ALU.mult, ALU.add)
                lnv = smp.tile([128, 1], f32, tag="lnv", name=f"lnv_{tt}")
                nc.scalar.activation(lnv[:], dvar[:], ACTF.Ln, bias=eps_ln[:],
                                     scale=1.0 / D)
                rsig = smp.tile([128, 1], f32, tag="rsig", name=f"rsig_{tt}")
                nc.scalar.activation(rsig[:], lnv[:], ACTF.Exp, bias=0.0, scale=-0.5)
                nmr = smp.tile([128, 1], f32, tag="nmr", name=f"nmr_{tt}")
                nc.gpsimd.tensor_scalar(nmr[:], s1[:], rsig[:], -1.0 / D,
                                        ALU.mult, ALU.mult)
                nc.vector.tensor_scalar(xn_t[:, 0:512], x_sb[:, 0:512],
                                        rsig[:], nmr[:], ALU.mult, ALU.add)
                nc.vector.tensor_scalar(xn_t[:, 512:D], x_sb[:, 512:D],
                                        rsig[:], nmr[:], ALU.mult, ALU.add)
                for half in range(2):
                    ps4 = tpp.tile([128, 4, 128], f32r, tag="tp",
                                   name=f"xt_{tt}_{half}")
                    for b in range(4):
                        dt_i = half * 4 + b
                        nc.tensor.transpose(
                            ps4[:, b, :], xn_t[:, dt_i * 128:(dt_i + 1) * 128],
                            id_sb[:])
                    nc.scalar.copy(
                        xnT[:, half * 4:(half + 1) * 4, ts], ps4[:])

            pend = {}

            def group_mm(grp, tt):
                """Matmul half of one qkv group token-tile."""
                w_halves = w_tiles[grp]
                ts = slice(tt * 128, (tt + 1) * 128)
                ps_q = psb.tile([128, 512], f32, tag="ps512")
                for dt_i in range(DT):
                    nc.tensor.matmul(ps_q[:], xnT[:, dt_i, ts],
                                     w_halves[dt_i // 2][:, dt_i % 2, :],
                                     start=(dt_i == 0), stop=(dt_i == DT - 1))
                return (tt, ps_q)

            def group_fin(grp, st):
                """Finish one group token-tile.
                v (4/5): bf16 copy into v_aug on DVE.
                q (0/1): square/reduce -> per-tile rsqrt (ACT Ln/Exp, tiny) ->
                         normalize -> XBAR transpose.
                k (2/3): square/reduce into ssk (rsqrt deferred to one batched
                         Ln/Exp per group; 1/|k| becomes the exp scale) ->
                         gamma multiply -> XBAR transpose of RAW k*gamma.
                Pre-attention groups (0/2) square on ACT straight from PSUM;
                mid-attention groups (1/3) stage PSUM via one DVE copy and
                square on Pool so nothing small ever blocks the exp stream."""
                kind = grp // 2
                chunk = grp % 2
                tt, ps_q = st
                ts = slice(tt * 128, (tt + 1) * 128)
                ps3 = ps_q.rearrange("p (h c) -> p h c", c=64)
                if kind == 2:
                    nc.vector.tensor_copy(
                        v_aug[:, tt, chunk * 8:chunk * 8 + 8, 0:64], ps3)
                    return
                sq = sqp.tile([128, 512], f32, tag="sq")
                sq3 = sq.rearrange("p (h c) -> p h c", c=64)
                if chunk == 0:
                    # pre-attention: DVE/Pool queues are shallow — read PSUM
                    # directly (square on ACT, downstream on DVE)
                    src3 = ps3
                    nc.scalar.activation(sq3, ps3, ACTF.Square,
                                         bias=0.0, scale=1.0)
                else:
                    # mid-attention: stage PSUM via one DVE copy so the psb
                    # slot frees fast; square on Pool keeps ACT exp-only
                    qf = qfp.tile([128, 512], f32, tag="qf")
                    nc.vector.tensor_copy(qf[:], ps_q[:])
                    src3 = qf.rearrange("p (h c) -> p h c", c=64)
                    nc.gpsimd.tensor_tensor(sq3, src3, src3, ALU.mult)
                if kind == 1:
                    nc.vector.tensor_reduce(ssk[chunk][:, tt, :], sq3,
                                            AX.X, ALU.add)
                    kg = kgp.tile([128, 512], bf16, tag="kg")
                    keng = nc.vector if chunk == 0 else nc.gpsimd
                    keng.tensor_tensor(
                        kg.rearrange("p (h c) -> p h c", c=64), src3,
                        gfull_sb[:, chunk * 512:(chunk + 1) * 512]
                        .rearrange("p (h c) -> p h c", c=64), ALU.mult)
                    nc.sync.dma_start_transpose(knT_ab[chunk][:, :, ts], kg[:])
                else:
                    ss = smp.tile([128, 8], f32, tag="ss")
                    nc.vector.tensor_reduce(ss[:], sq3, AX.X, ALU.add)
                    lnss = smp.tile([128, 8], f32, tag="lnss")
                    nc.scalar.activation(lnss[:], ss[:], ACTF.Ln,
                                         bias=eps_rms[:], scale=1.0)
                    rsq = smp.tile([128, 8], f32, tag="rsq")
                    nc.scalar.activation(rsq[:], lnss[:], ACTF.Exp,
                                         bias=0.0, scale=-0.5)
                    qn_t = stb.tile([128, 512], bf16, tag="qn_t")
                    nc.vector.tensor_tensor(
                        qn_t.rearrange("p (h c) -> p h c", c=64), src3,
                        rsq[:, :, None].to_broadcast((128, 8, 64)), ALU.mult)
                    nc.sync.dma_start_transpose(qnT_ab[chunk][:, :, ts], qn_t[:])

            def group_tt(grp, tt):
                """Software-pipelined group step: the finish half runs one
                token-tile behind the matmuls, so PE never waits on it."""
                if grp in pend:
                    group_fin(grp, pend.pop(grp))
                pend[grp] = group_mm(grp, tt)

            def group_flush(grp):
                if grp in pend:
                    group_fin(grp, pend.pop(grp))

            def rk_batch(chunk, lo=0, hi=NT):
                """Batched Ln + Exp for 1/|k| over token-tiles [lo, hi)."""
                lnk = smp.tile([128, NT, 8], f32, tag="lnk",
                               name=f"lnk_{chunk}_{lo}")
                nc.scalar.activation(lnk[:, lo:hi, :], ssk[chunk][:, lo:hi, :],
                                     ACTF.Ln, bias=eps_rms[:], scale=1.0)
                nc.scalar.activation(rk_all[chunk][:, lo:hi, :],
                                     lnk[:, lo:hi, :], ACTF.Exp,
                                     bias=0.0, scale=-0.5)

            def scores_jts(h, jt_lo, jt_hi):
                ch = h // 8
                hl = h % 8
                pc = hl // 2
                hp = slice((h % 2) * 64, (h % 2) * 64 + 64)
                qT = qnT_ab[ch]
                kT = knT_ab[ch]
                if h not in pts_box:
                    pts_box[h] = [
                        ptp.tile([128, N], bf16, tag="pT", name=f"pT_{h}_{jt}")
                        for jt in range(NT)]
                pts = pts_box[h]
                for jt in range(jt_lo, jt_hi):
                    ps_s = ps1024.tile([128, 1024], f32, tag="ps1024")
                    for ih in range(2):
                        nc.tensor.matmul(
                            ps_s[:, ih * 512:(ih + 1) * 512],
                            kT[hp, pc, jt * 128:(jt + 1) * 128],
                            qT[hp, pc, ih * 512:(ih + 1) * 512],
                            start=True, stop=True)
                    nc.scalar.activation(pts[jt][:], ps_s[:], ACTF.Exp,
                                         bias=0.0,
                                         scale=rk_all[ch][:, jt, hl:hl + 1])

            def scores_head(h):
                scores_jts(h, 0, NT)

            def attnv_half(h, half, pts):
                # attn@v flipped: out[i, d(+denom)] — stationary = prob chunk,
                # moving = v||1. 4 sub-bank accumulation groups per PSUM tile.
                ch = h // 8
                hl = h - 8 * ch
                osb_c = osb_box[ch]
                ps4 = tpp.tile([128, 4, 128], f32, tag="tp",
                               name=f"pso_{h}_{half}")
                # A multi-matmul accumulation group must own its PSUM bank,
                # so we cannot run 4 start..stop groups in this one-bank
                # tile. Instead every matmul is a self-contained group:
                # the first per range writes (start=True zeroes only its
                # own output range), the rest accumulate-write.
                for k in range(4):
                    it = half * 4 + k
                    for jt in range(NT):
                        nc.tensor.matmul(
                            ps4[0:128, k, 0:65],
                            pts[jt][:, it * 128:(it + 1) * 128],
                            v_aug[:, jt, h, 0:65],
                            start=(jt == 0), stop=True,
                            skip_group_check=True)
                nc.vector.tensor_copy(
                    osb_c[:, half * 4:(half + 1) * 4, hl, 0:65],
                    ps4[:, :, 0:65])

            def attnv_head(h):
                pts = pts_box.pop(h)
                attnv_half(h, 0, pts)
                attnv_half(h, 1, pts)

            def new_chunk(ch):
                osb_box[ch] = osbp.tile([128, NT, 8, 65], bf16, tag="osb",
                                        name=f"osb_{ch}")

            def finish_its(ch, its_range):
                """Batched reciprocal (DVE) + normalize (Pool) + XBAR
                transpose into ohn pairs [4ch .. 4ch+4)."""
                osb_c = osb_box[ch]
                for it in its_range:
                    its = slice(it * 128, (it + 1) * 128)
                    r_f = smp.tile([128, 8], f32, tag="rf", name=f"rf_{ch}_{it}")
                    nc.vector.reciprocal(
                        r_f[:], osb_c[:, it, :, 64:65].rearrange("p h o -> p (h o)"))
                    on_pk = onp.tile([128, 512], bf16, tag="onpk",
                                     name=f"onpk_{ch}_{it}")
                    on3 = on_pk.rearrange("p (h c) -> p h c", c=64)
                    if ch == 1:
                        # tail: split each normalize across Pool+DVE (both
                        # idle) — production rate bounds the proj-2 stream
                        nc.gpsimd.tensor_tensor(
                            on3[:, 0:4, :], osb_c[:, it, 0:4, 0:64],
                            r_f[:, 0:4][:, :, None].to_broadcast((128, 4, 64)),
                            ALU.mult)
                        nc.vector.tensor_tensor(
                            on3[:, 4:8, :], osb_c[:, it, 4:8, 0:64],
                            r_f[:, 4:8][:, :, None].to_broadcast((128, 4, 64)),
                            ALU.mult)
                    else:
                        nc.gpsimd.tensor_tensor(
                            on3, osb_c[:, it, :, 0:64],
                            r_f[:, :, None].to_broadcast((128, 8, 64)), ALU.mult)
                    nc.sync.dma_start_transpose(
                        ohn[:, 4 * ch:4 * ch + 4, its], on_pk[:])

            def proj1_it(it):
                """Out-proj pass 1: chunk-1 heads (pairs 0..3) into bf16
                partials — runs as chunk-2 PE filler."""
                if oacc_box[0] is None:
                    oacc_box[0] = bigp.tile([128, NT, 2, 512], bf16, tag="big",
                                            name="o_acc")
                o_acc = oacc_box[0]
                its = slice(it * 128, (it + 1) * 128)
                for dh in range(2):
                    ps_f = psb.tile([128, 512], f32, tag="ps512",
                                    name=f"ps1_{it}_{dh}")
                    for p in range(4):
                        nc.tensor.matmul(
                            ps_f[:], ohn[:, p, its],
                            wout_q[p // 2][:, p % 2, dh * 512:(dh + 1) * 512],
                            start=(p == 0), stop=(p == 3))
                    nc.vector.tensor_copy(o_acc[:, it, dh, :], ps_f[:])

            def proj2_it(it):
                """Out-proj pass 2: chunk-2 heads added to pass-1 partials;
                stores go out via the Pool SWDGE (keeps SP free for XBAR)."""
                o_acc = oacc_box[0]
                its = slice(it * 128, (it + 1) * 128)
                o_sb = osp.tile([128, 1024], bf16, tag="o_sb",
                                name=f"o_sb_{it}")
                for dh in range(2):
                    ps_f = psb.tile([128, 512], f32, tag="ps512",
                                    name=f"ps2_{it}_{dh}")
                    for p in range(4, 8):
                        nc.tensor.matmul(
                            ps_f[:], ohn[:, p, its],
                            wout_q[p // 2][:, p % 2, dh * 512:(dh + 1) * 512],
                            start=(p == 4), stop=(p == 7))
                    nc.vector.tensor_tensor(o_sb[:, dh * 512:(dh + 1) * 512],
                                            ps_f[:], o_acc[:, it, dh, :], ALU.add)
                nc.scalar.dma_start(out[its, :], o_sb[:])

            # ---------- Schedule ----------
            # LN first (PE: x transposes only), then q/k chunk-1 dense;
            # attention heads with two group-tiles between score half-blocks
            # and attnv of the previous head after; g3 woven into S6/S7 so the
            # chunk boundary needs no extra fill; chunk-2 heads use proj
            # pass-1 as filler; tail streams proj pass-2 against finish(1).
            for tt in range(NT):
                ln_tile(tt)
            for tt in range(NT):
                group_tt(0, tt)
                if tt in (2, 4, 6):
                    w_issue(1)
            group_flush(0)
            w_issue(1)
            for tt in range(NT):
                group_tt(2, tt)
                if tt in (2, 4, 6):
                    w_issue(1)
                if tt == 5:
                    rk_batch(0, 0, 4)
            group_flush(2)
            w_issue(1)
            rk_batch(0, 4, NT)
            new_chunk(0)

            WGRP = [4, 4, 1, 1, 3, 3, 5, 5]
            for h in range(8):
                g = WGRP[h]
                base = (h % 2) * 4
                scores_jts(h, 0, 4)
                group_tt(g, base + 0)
                group_tt(g, base + 1)
                if h >= 1:
                    w_issue(1)
                scores_jts(h, 4, 8)
                group_tt(g, base + 2)
                group_tt(g, base + 3)
                if h >= 1:
                    w_issue(1)
                if base + 3 == 7:
                    group_flush(g)
                    if g == 4:
                        nc.gpsimd.memset(v_aug[:, :, 0:8, 64:65], 1.0)
                    elif g == 5:
                        nc.gpsimd.memset(v_aug[:, :, 8:16, 64:65], 1.0)
                if h > 0:
                    attnv_head(h - 1)
            w_issue(1)
            # rk(1) issued here, two heads after the g3 fins completed, so
            # its Ln/Exp never wait at the ACT queue head
            rk_batch(1, 0, 4)
            rk_batch(1, 4, NT)
            scores_head(8)
            attnv_head(7)
            new_chunk(1)
            scores_head(9)
            attnv_head(8)
            finish_its(0, range(0, 2))
            scores_head(10)
            attnv_head(9)
            finish_its(0, range(2, 3))
            proj1_it(0)
            scores_head(11)
            attnv_head(10)
            finish_its(0, range(3, 4))
            proj1_it(1)
            scores_head(12)
            attnv_head(11)
            finish_its(0, range(4, 5))
            proj1_it(2)
            scores_head(13)
            attnv_head(12)
            finish_its(0, range(5, 6))
            proj1_it(3)
            scores_head(14)
            attnv_head(13)
            finish_its(0, range(6, 7))
            proj1_it(4)
            scores_head(15)
            attnv_head(14)
            finish_its(0, range(7, 8))
            proj1_it(5)
            proj1_it(6)
            pts15 = pts_box.pop(15)
            attnv_half(15, 0, pts15)
            proj1_it(7)
            finish_its(1, range(0, 4))
            attnv_half(15, 1, pts15)
            finish_its(1, range(4, 8))
            for it in range(NT):
                proj2_it(it)
    return nc


_NC_CACHE = None


def _patch_act_tables():
    """Steer bacc's greedy act-table-set selection to natural_log_exp_and_others
    for every function this kernel uses (exp/ln/square/copy/identity), by
    hiding those functions from all earlier sets. Set order (and thus the
    act_func_set_id each load emits) is unchanged, so the runtime tables are
    correct — but all our activations resolve to one co-resident set and the
    kernel performs a single table load instead of thrashing."""
    import collections
    import concourse.bacc as _bacc
    import concourse.hw_specs as _hw
    orig = getattr(_hw.get_activation_tables, '__wrapped_orig__', _hw.get_activation_tables)

    def patched(arch):
        d = orig(arch)
        key = "natural_log_exp_and_others"
        if key not in d:
            return d
        mine = d[key]
        hidden = {f for f in mine}
        nd = collections.OrderedDict()
        for k, v in d.items():
            if k == key:
                nd[k] = v
            else:
                nd[k] = v - hidden
        return nd
    patched.__wrapped_orig__ = orig
    _hw.get_activation_tables = patched
    _bacc.get_activation_tables = patched


def _get_nc():
    global _NC_CACHE
    if _NC_CACHE is None:
        _patch_act_tables()
        nc = build()
        nc.finalize()
        _NC_CACHE = nc
    return _NC_CACHE


def kernel(x, ln_gamma, q_gamma, k_gamma, w_qkv, w_out):
    x = np.asarray(x, dtype=np.float32)
    ln_gamma = np.asarray(ln_gamma, dtype=np.float32)
    q_gamma = np.asarray(q_gamma, dtype=np.float32).reshape(H, C)
    k_gamma = np.asarray(k_gamma, dtype=np.float32).reshape(H, C)
    w_qkv = np.asarray(w_qkv, dtype=np.float32)
    w_out = np.asarray(w_out, dtype=np.float32)

    wqkv_eff = ln_gamma[:, None] * w_qkv
    wqk = np.ascontiguousarray(wqkv_eff[:, 0:2048], dtype=np.float32)
    wv_bf = np.ascontiguousarray(wqkv_eff[:, 2048:], dtype=np.float32)
    wout_bf = w_out.astype(ml_dtypes.bfloat16)
    gfull = np.tile((64.0 * q_gamma * k_gamma).reshape(1, H * C),
                    (128, 1)).astype(ml_dtypes.bfloat16)
    ident = np.eye(128, dtype=np.float32)

    nc = _get_nc()
    in_maps = [
        {"x": np.ascontiguousarray(x[i]), "wqkv": wqk, "wv": wv_bf,
         "wout": wout_bf, "g": gfull, "ident": ident}
        for i in range(8)
    ]
    res = run_bass_kernel_spmd(nc, in_maps, core_ids=list(range(8)))
    return np.stack([np.asarray(res.results[i]["out"], dtype=np.float32)
                     for i in range(8)], axis=0)
